# revision 15
# baseline (speedup 1.0000x reference)
"""Trainium2 Bass kernel for nn_EstraNet_1443109012284.

Mathematical reduction: the reference's FAVOR+/trig branch (phi_q, aux_q/k,
fr_q/k, aux_A, A) does not feed the output.  The output is exactly

    out[b,n,d] = sum_{h,c} W_o[h,c,d] * norma[h] * sum_{d'} W_v[d',h,c] * x[b,n,d']
               = (x @ M)[b,n,d],   M[d',d] = sum_{h,c} W_v[d',h,c] norma[h] W_o[h,c,d]

with norma[h] = || sum_d s_p[h] W_p[d,h,:] beta_p[d] ||_2.

M is a tiny [512,512] matrix folded on the host; the device does the single
big GEMM  y[32768,512] = x[32768,512] @ M[512,512]  data-parallel over rows:
each of the 8 cores handles 4096 rows.

Device design (per core): compute yT = M.T-contracted x, i.e.
    yT[d, n] = sum_k M[k, d] * xT[k, n]
- lhsT (stationary) = M chunk [128k x 128d]  -> only 16 weight loads total,
  each reused for 8 back-to-back matmuls (same-weight MMs pipeline at
  N/2.4GHz; different-weight MMs pay a full array drain each).
- rhs (moving) = xT stripe [128k x 512n], fed pre-transposed from the host
  so no on-device transpose is needed.
- PSUM holds one full d-row-block sweep: 8 banks of [128, 512].
- Output is written as yT [512, 4096] contiguously; host transposes back.
"""

import os as _os
import sys

sys.path.insert(0, "/opt/trn_rl_repo")

import numpy as np

import concourse.bass as bass
import concourse.tile as tile
from concourse import bacc, mybir
from concourse.bass_utils import run_bass_kernel_spmd

N_CORES = 8
ROWS = 32768          # B*N = 8*4096
RPC = ROWS // N_CORES  # rows per core = 4096
D = 512
KC = 4                # contraction chunks of 128
NJ = RPC // 512       # moving chunks of 512 per sweep = 8
DT = D // 128         # output row-blocks = 4

# device compute dtype: "fp32" (exact, 4 cyc/row), "bf16" (1 cyc/row),
# "f32r" (fp32 data, reduced-precision fast path)
COMPUTE_DTYPE = _os.environ.get("KERNEL_DTYPE", "bf16")

_DT = {
    "fp32": mybir.dt.float32,
    "f32r": mybir.dt.float32r,
    "bf16": mybir.dt.bfloat16,
}


def _np_dtype(token):
    if token == "bf16":
        import ml_dtypes

        return ml_dtypes.bfloat16
    return np.float32


HB = 4                 # n-quarters per stripe
HW = RPC // HB         # 1024 columns per quarter
JH = HW // 512         # 2 moving chunks of 512 per phase
N_WARM = int(_os.environ.get("KERNEL_NWARM", "10"))            # dummy matmuls to burn the HAM cold-clock ramp


def _build(token):
    dt_in = _DT[token]
    nc = bacc.Bacc("TRN2", target_bir_lowering=False)
    # x pre-transposed, [k-chunk, quarter, 128, 1024] so each quarter-stripe
    # is one contiguous DMA
    xt = nc.dram_tensor("xt", [KC, HB, 128, HW], dt_in, kind="ExternalInput")
    mm = nc.dram_tensor("mm", [KC, 128, D], dt_in, kind="ExternalInput")
    yt = nc.dram_tensor("yt", [D, RPC], mybir.dt.float32, kind="ExternalOutput")

    with tile.TileContext(nc) as tc:
        with (
            tc.tile_pool(name="xp", bufs=1) as xp,
            tc.tile_pool(name="mp", bufs=1) as mp,
            tc.tile_pool(name="op", bufs=3) as op,
            tc.tile_pool(name="pp", bufs=6, space="PSUM") as pp,
        ):
            m_sb = mp.tile([128, KC, D], dt_in)
            nc.sync.dma_start(out=m_sb[:], in_=mm.rearrange("k p d -> p k d"))

            # PE warmup: matmuls on m_sb only (depends just on the tiny m
            # DMA) keep the PE busy while x streams in, so the HAM clock
            # ramp is paid during the DMA wait, not during real work
            warm = pp.tile([128, 512], mybir.dt.float32, tag="ps", name="warm")
            for w in range(N_WARM):
                nc.tensor.matmul(
                    warm[:], m_sb[:, 0, 0:128], m_sb[:, w % KC, :], start=True, stop=True
                )

            # load quarter-stripes in the order compute consumes them
            x_sb = {}
            for h in range(HB):
                for k in range(KC):
                    t = xp.tile([128, HW], dt_in, tag=f"x{k}{h}", name=f"x{k}{h}")
                    nc.sync.dma_start(out=t[:], in_=xt[k, h])
                    x_sb[(k, h)] = t

            # phases: h outer (first phase only needs the first 4 quarter
            # DMAs), d inner.  j-major MM order inside a phase so each PSUM
            # bank finishes early and its ACT copy + output DMA overlap the
            # rest of the phase.  Copies all on ACT: PE drain + a single
            # reader share PSUM without throttling the PE.
            for h in range(HB):
                for d in range(DT):
                    d0 = d * 128
                    ot = op.tile([128, HW], mybir.dt.float32, name="ot")
                    for j in range(JH):
                        ps = pp.tile(
                            [128, 512], mybir.dt.float32, tag="ps", name=f"ps_{h}_{d}_{j}"
                        )
                        for k in range(KC):
                            nc.tensor.matmul(
                                ps[:],
                                m_sb[:, k, d0 : d0 + 128],
                                x_sb[(k, h)][:, j * 512 : (j + 1) * 512],
                                start=(k == 0),
                                stop=(k == KC - 1),
                            )
                        nc.scalar.copy(ot[:, j * 512 : (j + 1) * 512], ps[:])
                    nc.sync.dma_start(out=yt[d0 : d0 + 128, h * HW : (h + 1) * HW], in_=ot[:])
    nc.compile()
    return nc


def _fold_m(W_v, s_p, W_p, beta_p, W_o):
    """Host-side constant folding of the tiny parameter tensors into M."""
    W_v = np.asarray(W_v, dtype=np.float64)
    s_p = np.asarray(s_p, dtype=np.float64)
    W_p = np.asarray(W_p, dtype=np.float64)
    beta_p = np.asarray(beta_p, dtype=np.float64)
    W_o = np.asarray(W_o, dtype=np.float64)
    phi = np.einsum("h,dhc,d->hc", s_p, W_p, beta_p)
    norma = np.linalg.norm(phi, axis=1)  # [h]
    M = np.einsum("dhc,h,hce->de", W_v, norma, W_o)  # [512, 512]
    return M.astype(np.float32)


_prog_cache = {}
_last_in_maps = None  # kept for test.py profiling reuse
_last_result = None


def _run(in_maps, token, **kwargs):
    if token not in _prog_cache:
        _prog_cache[token] = _build(token)
    return run_bass_kernel_spmd(_prog_cache[token], in_maps, list(range(N_CORES)), **kwargs)


def kernel(x, W_v, s_p, c_p, W_p, W_A, W_o, beta_p, beta_i_p, **_unused):
    global _last_in_maps, _last_result
    token = COMPUTE_DTYPE
    np_dt = _np_dtype(token)

    x = np.asarray(x, dtype=np.float32)
    M = _fold_m(W_v, s_p, W_p, beta_p, W_o)

    B, N, Dd = x.shape
    assert B * N == ROWS and Dd == D, (x.shape,)

    mmc = np.ascontiguousarray(M.reshape(KC, 128, D)).astype(np_dt)
    xf = x.reshape(ROWS, D)

    in_maps = []
    for c in range(N_CORES):
        sh = xf[c * RPC : (c + 1) * RPC]               # [4096, 512]
        xT = sh.T.astype(np_dt)                        # [512, 4096]
        # [KC, 128, HB, HW] -> [KC, HB, 128, HW], each half-stripe contiguous
        xs = np.ascontiguousarray(
            xT.reshape(KC, 128, HB, HW).transpose(0, 2, 1, 3)
        )
        in_maps.append({"xt": xs, "mm": mmc})

    _last_in_maps = in_maps
    res = _run(in_maps, token)
    _last_result = res
    out = np.empty((ROWS, D), dtype=np.float32)
    for c in range(N_CORES):
        out[c * RPC : (c + 1) * RPC] = res.results[c]["yt"].T
    return out.reshape(B, N, D)


if __name__ == "__main__":
    # smoke test with random data
    rng = np.random.default_rng(0)
    x = rng.standard_normal((8, 4096, 512)).astype(np.float32)
    W_v = rng.standard_normal((512, 8, 64)).astype(np.float32) * 0.01
    s_p = np.ones((8,), np.float32)
    c_p = np.ones((8,), np.float32)
    W_p = rng.standard_normal((512, 8, 64)).astype(np.float32) * 0.01
    W_A = rng.standard_normal((256, 64)).astype(np.float32)
    W_o = rng.standard_normal((8, 64, 512)).astype(np.float32) * 0.01
    beta_p = rng.standard_normal((512,)).astype(np.float32) * 1e-5
    beta_i_p = rng.standard_normal((4096, 512)).astype(np.float32) * 1e-5
    out = kernel(x, W_v=W_v, s_p=s_p, c_p=c_p, W_p=W_p, W_A=W_A, W_o=W_o,
                 beta_p=beta_p, beta_i_p=beta_i_p)
    M = _fold_m(W_v, s_p, W_p, beta_p, W_o)
    exp = (x.reshape(-1, 512).astype(np.float64) @ M.astype(np.float64)).reshape(8, 4096, 512)
    err = np.abs(out - exp).max() / (np.abs(exp).max() + 1e-30)
    print("smoke rel err:", err)


# revision 16
# speedup vs baseline: 1.1566x; 1.1566x over previous
"""Trainium2 Bass kernel for nn_EstraNet_1443109012284.

Mathematical reduction: the reference's FAVOR+/trig branch (phi_q, aux_q/k,
fr_q/k, aux_A, A) does not feed the output.  The output is exactly

    out[b,n,d] = sum_{h,c} W_o[h,c,d] * norma[h] * sum_{d'} W_v[d',h,c] * x[b,n,d']
               = (x @ M)[b,n,d],   M[d',d] = sum_{h,c} W_v[d',h,c] norma[h] W_o[h,c,d]

with norma[h] = || sum_d s_p[h] W_p[d,h,:] beta_p[d] ||_2.

M is a tiny [512,512] matrix folded on the host; the device does the single
big GEMM  y[32768,512] = x[32768,512] @ M[512,512]  data-parallel over rows:
each of the 8 cores handles 4096 rows.

Device design (per core): compute yT[d, n] = sum_k M[k, d] * xT[k, n]
- lhsT (stationary) = M chunk [128k x 128d]; rhs (moving) = xT quarter
  [128k x 512n], fed pre-transposed from the host (no on-device transpose).
- Same/alternating-weight MMs pipeline at 512/2.4GHz = 216 ns.
- PSUM->SBUF copies all on ONE engine (ACT): PE drain + a single reader
  share PSUM fine; two concurrent readers throttle the PE ~2.3x.
- PE warmed up with dummy matmuls (dep: a memset tile only) during the
  input-DMA window so the HAM clock ramp doesn't tax real work.
- fp16 path (default): x, M, y all fp16, M pre-scaled by an exact power of
  two so M / y avoid the fp16 subnormal range; host multiplies the scale
  back out.  fp16 keeps 10 mantissa bits (vs bf16's 7) and halves output
  DMA vs fp32 -> kernel is PE-bound at ~216ns per [128x128]x[128x512] MM.
"""

import os as _os
import sys

sys.path.insert(0, "/opt/trn_rl_repo")

import numpy as np

import concourse.bass as bass
import concourse.tile as tile
from concourse import bacc, mybir
from concourse.bass_utils import run_bass_kernel_spmd

N_CORES = 8
ROWS = 32768           # B*N = 8*4096
RPC = ROWS // N_CORES  # rows per core = 4096
D = 512
KC = 4                 # contraction chunks of 128
DT = D // 128          # output row-blocks = 4
HB = 4                 # n-quarters per stripe
HW = RPC // HB         # 1024 columns per quarter
JH = HW // 512         # moving chunks of 512 per phase = 2

COMPUTE_DTYPE = _os.environ.get("KERNEL_DTYPE", "fp16")
N_WARM = int(_os.environ.get("KERNEL_NWARM", "10"))

_DT = {
    "fp32": mybir.dt.float32,
    "f32r": mybir.dt.float32r,
    "bf16": mybir.dt.bfloat16,
    "fp16": mybir.dt.float16,
}


def _np_dtype(token):
    if token == "bf16":
        import ml_dtypes

        return ml_dtypes.bfloat16
    if token == "fp16":
        return np.float16
    return np.float32


def _build(token):
    dt_in = _DT[token]
    dt_out = mybir.dt.float16 if token == "fp16" else mybir.dt.float32
    nc = bacc.Bacc("TRN2", target_bir_lowering=False)
    # x pre-transposed, [k-chunk, quarter, 128, 1024]: each quarter-stripe is
    # one contiguous DMA
    xt = nc.dram_tensor("xt", [KC, HB, 128, HW], dt_in, kind="ExternalInput")
    mm = nc.dram_tensor("mm", [KC, 128, D], dt_in, kind="ExternalInput")
    yt = nc.dram_tensor("yt", [D, RPC], dt_out, kind="ExternalOutput")

    with tile.TileContext(nc) as tc:
        with (
            tc.tile_pool(name="xp", bufs=1) as xp,
            tc.tile_pool(name="mp", bufs=1) as mp,
            tc.tile_pool(name="op", bufs=4) as op,
            tc.tile_pool(name="pp", bufs=7, space="PSUM") as pp,
            tc.tile_pool(name="wp", bufs=1, space="PSUM") as wp,
        ):
            # PE warmup: depends only on a memset tile, so it starts at
            # ~6us (right after engine code load) and burns the HAM
            # cold-clock ramp while the x DMAs are still in flight.
            wz = mp.tile([128, 512], dt_in, name="wz")
            nc.gpsimd.memset(wz[:], 1.0)
            warm = wp.tile([128, 512], mybir.dt.float32, name="warm")
            for w in range(N_WARM):
                nc.tensor.matmul(
                    warm[:], wz[:, 0:128], wz[:], start=True, stop=True
                )

            # m on the scalar HWDGE queue, x quarters on the sync queue:
            # the two issue streams run in parallel on different sequencers
            m_sb = mp.tile([128, KC, D], dt_in, name="m_sb")
            nc.scalar.dma_start(out=m_sb[:], in_=mm.rearrange("k p d -> p k d"))

            x_sb = {}
            for h in range(HB):
                for k in range(KC):
                    t = xp.tile([128, HW], dt_in, tag=f"x{k}{h}", name=f"x{k}{h}")
                    eng = nc.sync if (h * KC + k) % 2 == 0 else nc.scalar
                    eng.dma_start(out=t[:], in_=xt[k, h])
                    x_sb[(k, h)] = t

            # phases: h outer (first phase only needs the first 4 quarter
            # DMAs), d inner.  k-major MM order (4 weight switches per
            # phase, banks finish staggered); last phase j-major with per-
            # bank copy+DMA so the tail is short.
            NPH = HB * DT
            for ph in range(NPH):
                h, d = divmod(ph, DT)
                d0 = d * 128
                last = ph == NPH - 1
                ot = op.tile([128, HW], dt_out, name=f"ot{ph}", tag="ot")
                pss = [
                    pp.tile([128, 512], mybir.dt.float32, tag="ps", name=f"ps_{h}_{d}_{j}")
                    for j in range(JH)
                ]
                if last:
                    for j in range(JH):
                        for k in range(KC):
                            nc.tensor.matmul(
                                pss[j][:],
                                m_sb[:, k, d0 : d0 + 128],
                                x_sb[(k, h)][:, j * 512 : (j + 1) * 512],
                                start=(k == 0),
                                stop=(k == KC - 1),
                            )
                        nc.scalar.copy(ot[:, j * 512 : (j + 1) * 512], pss[j][:])
                        nc.sync.dma_start(
                            out=yt[d0 : d0 + 128, h * HW + j * 512 : h * HW + (j + 1) * 512],
                            in_=ot[:, j * 512 : (j + 1) * 512],
                        )
                else:
                    for k in range(KC):
                        for j in range(JH):
                            nc.tensor.matmul(
                                pss[j][:],
                                m_sb[:, k, d0 : d0 + 128],
                                x_sb[(k, h)][:, j * 512 : (j + 1) * 512],
                                start=(k == 0),
                                stop=(k == KC - 1),
                            )
                    for j in range(JH):
                        nc.scalar.copy(ot[:, j * 512 : (j + 1) * 512], pss[j][:])
                    nc.sync.dma_start(
                        out=yt[d0 : d0 + 128, h * HW : (h + 1) * HW], in_=ot[:]
                    )
    nc.compile()
    return nc


def _fold_m(W_v, s_p, W_p, beta_p, W_o):
    """Host-side constant folding of the tiny parameter tensors into M."""
    W_v = np.asarray(W_v, dtype=np.float64)
    s_p = np.asarray(s_p, dtype=np.float64)
    W_p = np.asarray(W_p, dtype=np.float64)
    beta_p = np.asarray(beta_p, dtype=np.float64)
    W_o = np.asarray(W_o, dtype=np.float64)
    phi = np.einsum("h,dhc,d->hc", s_p, W_p, beta_p)
    norma = np.linalg.norm(phi, axis=1)  # [h]
    M = np.einsum("dhc,h,hce->de", W_v, norma, W_o)  # [512, 512]
    return M.astype(np.float32)


_prog_cache = {}
_last_in_maps = None  # kept for test.py profiling reuse
_last_result = None


def _run(in_maps, token, **kwargs):
    if token not in _prog_cache:
        _prog_cache[token] = _build(token)
    return run_bass_kernel_spmd(_prog_cache[token], in_maps, list(range(N_CORES)), **kwargs)


def kernel(x, W_v, s_p, c_p, W_p, W_A, W_o, beta_p, beta_i_p, **_unused):
    global _last_in_maps, _last_result
    token = COMPUTE_DTYPE
    np_dt = _np_dtype(token)

    x = np.asarray(x, dtype=np.float32)
    M = _fold_m(W_v, s_p, W_p, beta_p, W_o)

    # fp16 path: scale M by an exact power of two so M entries and y values
    # sit in fp16 normal range; undo on the host after the run
    out_unscale = 1.0
    if token == "fp16":
        amax = float(np.abs(M).max())
        if amax > 0:
            e = int(np.floor(-np.log2(amax)))
            M = M * np.float32(2.0**e)
            out_unscale = 2.0**-e

    B, N, Dd = x.shape
    assert B * N == ROWS and Dd == D, (x.shape,)

    mmc = np.ascontiguousarray(M.reshape(KC, 128, D)).astype(np_dt)
    xf = x.reshape(ROWS, D)

    in_maps = []
    for c in range(N_CORES):
        sh = xf[c * RPC : (c + 1) * RPC]               # [4096, 512]
        xT = sh.T.astype(np_dt)                        # [512, 4096]
        # [KC, 128, HB, HW] -> [KC, HB, 128, HW], each quarter contiguous
        xs = np.ascontiguousarray(
            xT.reshape(KC, 128, HB, HW).transpose(0, 2, 1, 3)
        )
        in_maps.append({"xt": xs, "mm": mmc})

    _last_in_maps = in_maps
    res = _run(in_maps, token)
    _last_result = res
    out = np.empty((ROWS, D), dtype=np.float32)
    for c in range(N_CORES):
        yc = res.results[c]["yt"].astype(np.float32)
        if out_unscale != 1.0:
            yc *= np.float32(out_unscale)
        out[c * RPC : (c + 1) * RPC] = yc.T
    return out.reshape(B, N, D)


if __name__ == "__main__":
    # smoke test with random data
    rng = np.random.default_rng(0)
    x = rng.standard_normal((8, 4096, 512)).astype(np.float32)
    W_v = rng.standard_normal((512, 8, 64)).astype(np.float32) * 0.01
    s_p = np.ones((8,), np.float32)
    c_p = np.ones((8,), np.float32)
    W_p = rng.standard_normal((512, 8, 64)).astype(np.float32) * 0.01
    W_A = rng.standard_normal((256, 64)).astype(np.float32)
    W_o = rng.standard_normal((8, 64, 512)).astype(np.float32) * 0.01
    beta_p = rng.standard_normal((512,)).astype(np.float32) * 1e-5
    beta_i_p = rng.standard_normal((4096, 512)).astype(np.float32) * 1e-5
    out = kernel(x, W_v=W_v, s_p=s_p, c_p=c_p, W_p=W_p, W_A=W_A, W_o=W_o,
                 beta_p=beta_p, beta_i_p=beta_i_p)
    M = _fold_m(W_v, s_p, W_p, beta_p, W_o)
    exp = (x.reshape(-1, 512).astype(np.float64) @ M.astype(np.float64)).reshape(8, 4096, 512)
    err = np.abs(out - exp).max() / (np.abs(exp).max() + 1e-30)
    print("smoke rel err:", err)
